# revision 18
# baseline (speedup 1.0000x reference)
"""Trainium2 Bass kernel for nn_Classifier (25-step LIF SNN, 784->64->64->10).

Strategy (pure data parallel over batch, 8 cores, Bloc=2048 rows/core):

Host side (numpy, inside kernel()): shard batch, pre-transpose x -> xT with
row-pair column ordering, build layout constants (W1^T, block-diagonal
W2^T / W3^T split into exact bf16 triples, identity, packed biases).

Device side per core, "row-pair packed feature-major" layout for layers
1/2: tensors are [128 partitions = 2 row-parities x 64 features, 1024
cols], where packed column c holds local batch rows (2c, 2c+1) - parity
on the partition blocks. This makes every transposed output partition
own two CONSECUTIVE batch rows, so output DMA runs are 512B contiguous
(full DMA-engine line rate instead of the sub-512B read-modify-write
penalty).

  phase 1:  x1 = W1 @ xT + b1 on the PE over 7 K-chunks (fp32), bias
            fused into the PSUM->SBUF copy on the scalar engine.
  step loop (t = 0..24), split into 2 column-chunks of 512:
    L1: m1' = beta*m1'' + x1 in ONE DVE scalar_tensor_tensor (bit-exact
        vs the reference), s1 = m1' > 1 (DVE, bf16 0/1), m1'' = m1' - s1.
    L2: q2 = W2bd @ s1 as three exact bf16 matmuls (spikes are binary so
        every product is exact; W2hi+W2mid+W2lo == W2 exactly in fp32),
        m2 = beta*m2'' + q2 (DVE STT), +b2 on ACT, spike + reset as L1.
    L3: q3 = W3bd @ s2 (bf16 triple), +b3 in the ACT PSUM->SBUF copy,
        transposed to batch-major, LIF recurrence on DVE.
    Outputs: m1'/m2' transposed to batch-major via PE transpose-mode;
    spikes (DVE is_gt) and sigmoids (ACT) stream to DRAM per step.
"""

from contextlib import ExitStack

import numpy as np

import concourse.bass as bass
import concourse.bacc as bacc
import concourse.mybir as mybir
import concourse.tile as tile

F32 = mybir.dt.float32
BF16 = mybir.dt.bfloat16
BETA = 0.95
THR = 1.0

# full-size problem constants
B_FULL = 16384
N_CORES = 8
KIN_FULL = 784
F_HID = 64
O_OUT = 10
STEPS_FULL = 25


class Cfg:
    def __init__(self, Bloc, Kin, steps, CH, out_steps=None):
        self.Bloc = Bloc          # local batch rows per core
        self.Kin = Kin            # input features (784)
        self.steps = steps
        # DRAM output depth; steps beyond this wrap (timing builds only)
        self.out_steps = out_steps or steps
        self.G = 2                # row-parity groups on partitions
        self.F = F_HID
        self.O = O_OUT
        self.P = self.G * self.F  # 128
        self.PO = self.G * self.O  # 20
        assert Bloc % self.G == 0
        self.COLS = Bloc // self.G   # packed columns
        self.CH = CH                 # column chunk per matmul/psum tile
        assert self.COLS % CH == 0
        self.NCH = self.COLS // CH
        self.TP = 128                # transpose block width (input cols)
        assert CH % self.TP == 0
        self.TC = CH // self.TP      # transpose blocks per chunk
        # K chunking for phase 1
        self.kcs = []
        k = Kin
        while k > 0:
            c = min(128, k)
            self.kcs.append(c)
            k -= c


def build_nc(cfg: Cfg):
    """Builds the per-core SPMD Bass program. Returns (nc, out_names)."""
    G, F, O, P, PO = cfg.G, cfg.F, cfg.O, cfg.P, cfg.PO
    COLS, CH, NCH, TP, TC = cfg.COLS, cfg.CH, cfg.NCH, cfg.TP, cfg.TC
    steps, Kin, Bloc = cfg.steps, cfg.Kin, cfg.Bloc
    GF = G * F
    GO = G * O
    AL = mybir.AluOpType
    SIG = mybir.ActivationFunctionType.Sigmoid

    nc = bacc.Bacc("TRN2", target_bir_lowering=False, debug=False,
                   enable_asserts=False)

    # ---- DRAM parameters (per core) ----
    xT_d = nc.declare_dram_parameter("xT", [Kin, Bloc], F32, isOutput=False)
    w1t_d = nc.declare_dram_parameter("w1t", [Kin, F], F32, isOutput=False)
    w2s_d = [nc.declare_dram_parameter(f"w2{s}", [P, P], BF16, isOutput=False)
             for s in "abc"]
    w3s_d = [nc.declare_dram_parameter(f"w3{s}", [P, PO], BF16, isOutput=False)
             for s in "abc"]
    eye_d = nc.declare_dram_parameter("eye", [P, P], F32, isOutput=False)
    b1_d = nc.declare_dram_parameter("b1p", [P, 1], F32, isOutput=False)
    b2s_d = [nc.declare_dram_parameter(f"b2{s}", [1, P], BF16, isOutput=False)
             for s in "abc"]
    b3_d = nc.declare_dram_parameter("b3p", [PO, 1], F32, isOutput=False)

    osteps = cfg.out_steps
    spko_d = nc.declare_dram_parameter("spk_out", [osteps, Bloc, O], F32, isOutput=True)
    spk1_d = nc.declare_dram_parameter("spk1", [osteps, Bloc, F], F32, isOutput=True)
    spk2_d = nc.declare_dram_parameter("spk2", [osteps, Bloc, F], F32, isOutput=True)
    ss1_d = nc.declare_dram_parameter("ss1", [osteps, Bloc, F], F32, isOutput=True)
    ss2_d = nc.declare_dram_parameter("ss2", [osteps, Bloc, F], F32, isOutput=True)
    ss3_d = nc.declare_dram_parameter("ss3", [osteps, Bloc, O], F32, isOutput=True)

    # batch-major DRAM views. Packed column c = j*TP + u (j = global
    # 128-col block) of parity g maps to local batch row r = 2*c + g, so
    # each transposed output partition owns two consecutive DRAM rows:
    # per-t APs are [c][j][(g f)] with a 2*f contiguous (512B) inner run.
    NJ = NCH * TC
    def bview(d):
        return d[:].rearrange("t (j c g) f -> t c j (g f)",
                              j=NJ, c=TP, g=G)

    v_spk1, v_spk2, v_ss1, v_ss2 = map(bview, (spk1_d, spk2_d, ss1_d, ss2_d))
    v_spko, v_ss3 = map(bview, (spko_d, ss3_d))

    with tile.TileContext(nc) as tc, ExitStack() as es:
        cpool = es.enter_context(tc.tile_pool(name="const", bufs=1))
        eye_s = cpool.tile([P, P], F32, tag="eye")
        w2s_s = [cpool.tile([P, P], BF16, name=f"w2s{i}", tag=f"w2{i}")
                 for i in range(3)]
        w3s_s = [cpool.tile([P, PO], BF16, name=f"w3s{i}", tag=f"w3{i}")
                 for i in range(3)]
        b1_s = cpool.tile([P, 1], F32, tag="b1")
        b2s_s = [cpool.tile([1, P], BF16, name=f"b2s{i}", tag=f"b2{i}")
                 for i in range(3)]
        ones_s = cpool.tile([1, CH], BF16, tag="ones")
        b3_s = cpool.tile([PO, 1], F32, tag="b3")
        nthr_s = cpool.tile([P, 1], F32, tag="nthr")
        x1f = cpool.tile([P, COLS], F32, tag="x1f")
        m3sb = cpool.tile([TP, NCH * TC * GO], F32, tag="m3sb")
        loads = [(eye_s, eye_d), (b1_s, b1_d), (b3_s, b3_d)]
        loads += list(zip(w2s_s, w2s_d)) + list(zip(w3s_s, w3s_d))
        loads += list(zip(b2s_s, b2s_d))
        for t_s, t_d in loads:
            nc.sync.dma_start(t_s[:], t_d[:])
        nc.vector.memset(nthr_s[:], -THR)
        nc.vector.memset(ones_s[:], 1.0)

        # ---------------- phase 1: x1 = W1 @ xT + b1 ----------------
        with tc.tile_pool(name="ph1", bufs=1) as xp, \
             tc.tile_pool(name="ph1ps", bufs=2, space="PSUM") as pp:
            xts, w1ts = [], []
            koff = 0
            for i, kc in enumerate(cfg.kcs):
                xt_t = xp.tile([kc, Bloc], F32, tag=f"xt{i}")
                nc.sync.dma_start(xt_t[:], xT_d[koff:koff + kc, :])
                w1_t = xp.tile([kc, F], F32, tag=f"w1{i}")
                nc.sync.dma_start(w1_t[:], w1t_d[koff:koff + kc, :])
                xts.append(xt_t)
                w1ts.append(w1_t)
                koff += kc
            for j in range(NCH):
                ps = pp.tile([P, CH], F32, tag="x1ps")
                for g in range(G):
                    for i, kc in enumerate(cfg.kcs):
                        # host pre-orders xT columns as [block j][parity g][u]
                        xv = xts[i][:].rearrange("p (j g u) -> p j g u",
                                                 g=G, u=TP)
                        nc.tensor.matmul(
                            ps[g * F:(g + 1) * F, :], w1ts[i][:],
                            xv[:, j * TC:(j + 1) * TC, g, :],
                            start=(i == 0), stop=(i == len(cfg.kcs) - 1))
                nc.scalar.add(x1f[:, j * CH:(j + 1) * CH], ps[:], b1_s[:])

        # ---------------- state pools ----------------
        sp = es.enter_context(tc.tile_pool(name="state", bufs=3))
        bp = es.enter_context(tc.tile_pool(name="bside", bufs=4))
        pq2 = es.enter_context(tc.tile_pool(name="pq2", bufs=2, space="PSUM"))
        pq3 = es.enter_context(tc.tile_pool(name="pq3", bufs=1, space="PSUM"))
        pt12 = es.enter_context(tc.tile_pool(name="pt12", bufs=2, space="PSUM"))
        pt3 = es.enter_context(tc.tile_pool(name="pt3", bufs=1, space="PSUM"))

        mm1_prev = sp.tile([P, COLS], F32, tag="mm1")
        mm2_prev = sp.tile([P, COLS], F32, tag="mm2")
        for z in (mm1_prev, mm2_prev):
            nc.vector.memset(z[:], 0.0)
        nc.vector.memset(m3sb[:], 0.0)

        for t_ in range(steps):
            t = t_ % osteps
            mm1_new = sp.tile([P, COLS], F32, tag="mm1")
            mm2_new = sp.tile([P, COLS], F32, tag="mm2")
            s1b = bp.tile([TP, NJ * GF], F32, tag="s1b", name="s1b")
            ss1b = bp.tile([TP, NJ * GF], F32, tag="ss1b", name="ss1b")
            s2b = bp.tile([TP, NJ * GF], F32, tag="s2b", name="s2b")
            ss2b = bp.tile([TP, NJ * GF], F32, tag="ss2b", name="ss2b")
            s3b = bp.tile([TP, NJ * GO], F32, tag="s3b", name="s3b")
            ss3b = bp.tile([TP, NJ * GO], F32, tag="ss3b", name="ss3b")
            tb1 = pt12.tile([TP, NJ * P], F32, tag="pt12", name="tb1")
            tb2 = pt12.tile([TP, NJ * P], F32, tag="pt12", name="tb2")
            t3 = pt3.tile([TP, NJ * PO], F32, tag="pt3", name="t3")
            for h in range(NCH):
                cs = slice(h * CH, (h + 1) * CH)
                # ---- L1 membrane: m1' = beta*m1'' + x1  (DVE, bit-exact)
                m1p = bp.tile([P, CH], F32, tag="m1p", name="m1p")
                nc.vector.scalar_tensor_tensor(
                    m1p[:], mm1_prev[:, cs], BETA, x1f[:, cs],
                    AL.mult, AL.add)
                s1f = bp.tile([P, CH], BF16, tag="s1f", name="s1f")
                nc.vector.tensor_scalar(s1f[:], m1p[:], THR, None, AL.is_gt)
                nc.gpsimd.tensor_tensor(mm1_new[:, cs], m1p[:], s1f[:],
                                        AL.subtract)
                # ---- L2: q2 = W2 @ s1 (three exact bf16 passes)
                q2 = pq2.tile([P, CH], F32, tag="q2", name="q2")
                for i in range(3):
                    nc.tensor.matmul(q2[:], w2s_s[i][:], s1f[:],
                                     start=(i == 0), stop=False)
                for i in range(3):
                    nc.tensor.matmul(q2[:], b2s_s[i][:], ones_s[:],
                                     start=False, stop=(i == 2))
                m2p = bp.tile([P, CH], F32, tag="m2p", name="m2p")
                nc.vector.scalar_tensor_tensor(
                    m2p[:], mm2_prev[:, cs], BETA, q2[:], AL.mult, AL.add)
                s2f = bp.tile([P, CH], BF16, tag="s2f", name="s2f")
                nc.vector.tensor_scalar(s2f[:], m2p[:], THR, None, AL.is_gt)
                nc.gpsimd.tensor_tensor(mm2_new[:, cs], m2p[:], s2f[:],
                                        AL.subtract)
                # ---- L3 feed-forward: q3 = W3 @ s2 (+b3 in ACT copy)
                q3 = pq3.tile([PO, CH], F32, tag="q3", name="q3")
                for i in range(3):
                    nc.tensor.matmul(q3[:], w3s_s[i][:], s2f[:],
                                     start=(i == 0), stop=(i == 2))
                h3s = bp.tile([PO, CH], F32, tag="h3sb", name="h3s")
                nc.scalar.add(h3s[:], q3[:], b3_s[:])

                # ---- transposes into the per-step batch-major PSUM tiles
                for k in range(TC):
                    j = h * TC + k
                    nc.tensor.transpose(
                        tb1[:, j * P:(j + 1) * P],
                        m1p[:, k * TP:(k + 1) * TP], eye_s[:])
                    nc.tensor.transpose(
                        tb2[:, j * P:(j + 1) * P],
                        m2p[:, k * TP:(k + 1) * TP], eye_s[:])
                    nc.tensor.transpose(
                        t3[:, j * PO:(j + 1) * PO],
                        h3s[:, k * TP:(k + 1) * TP], eye_s[0:PO, 0:PO])

                # ---- L3 batch-major LIF (per half: columns of t3)
                ho = slice(h * TC * GO, (h + 1) * TC * GO)
                slab = m3sb[:, ho]
                m3t = bp.tile([TP, TC * GO], F32, tag="m3t", name="m3t")
                nc.vector.scalar_tensor_tensor(
                    m3t[:], slab, BETA,
                    t3[:, h * TC * PO:(h + 1) * TC * PO], AL.mult, AL.add)
                nc.gpsimd.tensor_scalar(s3b[:, ho], m3t[:], THR, None,
                                        AL.is_gt)
                nc.scalar.activation(ss3b[:, ho], m3t[:], SIG,
                                     bias=nthr_s[:])
                nc.gpsimd.tensor_tensor(slab, m3t[:], s3b[:, ho],
                                        AL.subtract)

            # ---- per-step batch-major spikes / sigmoids + one DMA each
            nc.vector.tensor_scalar(s1b[:], tb1[:], THR, None, AL.is_gt)
            nc.scalar.activation(ss1b[:], tb1[:], SIG, bias=nthr_s[:])
            nc.vector.tensor_scalar(s2b[:], tb2[:], THR, None, AL.is_gt)
            nc.scalar.activation(ss2b[:], tb2[:], SIG, bias=nthr_s[:])
            nc.sync.dma_start(v_spk1[t], s1b[:])
            nc.sync.dma_start(v_ss1[t], ss1b[:])
            nc.sync.dma_start(v_spk2[t], s2b[:])
            nc.sync.dma_start(v_ss2[t], ss2b[:])
            nc.sync.dma_start(v_spko[t], s3b[:])
            nc.sync.dma_start(v_ss3[t], ss3b[:])
            mm1_prev, mm2_prev = mm1_new, mm2_new

    nc.compile()
    out_names = ["spk_out", "spk1", "spk2", "ss1", "ss2", "ss3"]
    return nc, out_names


def _bf16_triple(w):
    """Exact 3-term bf16 decomposition of fp32 w (w == a + b + c in fp32)."""
    import ml_dtypes
    f32 = np.float32
    bf = ml_dtypes.bfloat16
    a = w.astype(bf)
    b = (w - a.astype(f32)).astype(bf)
    c = (w - a.astype(f32) - b.astype(f32)).astype(bf)
    assert ((a.astype(f32) + b.astype(f32) + c.astype(f32)) == w).all(), \
        "bf16 triple not exact"
    return a, b, c


def host_inputs(cfg: Cfg, x_shard, W1, b1, W2, b2, W3, b3):
    """Builds the per-core input map (numpy data marshalling only)."""
    G, F, O, P, PO, TP = cfg.G, cfg.F, cfg.O, cfg.P, cfg.PO, cfg.TP
    f32 = np.float32
    x_shard = np.asarray(x_shard, f32)
    # column order [block j][parity g][u]: column (j*G+g)*TP+u holds batch
    # row 2*(j*TP+u)+g
    J = cfg.COLS // TP
    j = np.arange(J)[:, None, None]
    g = np.arange(G)[None, :, None]
    u = np.arange(TP)[None, None, :]
    idx = (2 * (j * TP + u) + g).reshape(-1)
    xT = np.ascontiguousarray(x_shard[idx].T)                # [Kin, Bloc]
    w1t = np.ascontiguousarray(np.asarray(W1, f32).T)        # [Kin, F]
    w2bd = np.zeros((P, P), f32)
    w2bd[0:F, 0:F] = np.asarray(W2, f32).T
    w2bd[F:P, F:P] = np.asarray(W2, f32).T
    w3bd = np.zeros((P, PO), f32)
    w3bd[0:F, 0:O] = np.asarray(W3, f32).T
    w3bd[F:P, O:PO] = np.asarray(W3, f32).T
    b2p = np.tile(np.asarray(b2, f32), G).reshape(1, P)
    b2a, b2b, b2c = _bf16_triple(b2p)
    w2a, w2b, w2c = _bf16_triple(w2bd)
    w3a, w3b, w3c = _bf16_triple(w3bd)
    return {
        "xT": xT,
        "w1t": w1t,
        "w2a": w2a, "w2b": w2b, "w2c": w2c,
        "b2a": b2a, "b2b": b2b, "b2c": b2c,
        "w3a": w3a, "w3b": w3b, "w3c": w3c,
        "eye": np.eye(P, dtype=f32),
        "b1p": np.tile(np.asarray(b1, f32), G).reshape(P, 1),

        "b3p": np.tile(np.asarray(b3, f32), G).reshape(PO, 1),
    }


_CACHE = {}


def kernel_with_results(x, W1, b1, W2, b2, W3, b3, **run_kwargs):
    from concourse.bass_utils import run_bass_kernel_spmd

    x = np.asarray(x, np.float32)
    B = x.shape[0]
    assert B == B_FULL and x.shape[1] == KIN_FULL
    Bloc = B // N_CORES
    cfg = Cfg(Bloc=Bloc, Kin=KIN_FULL, steps=STEPS_FULL, CH=512)

    key = "full"
    if key not in _CACHE:
        _CACHE[key] = build_nc(cfg)
    nc, out_names = _CACHE[key]

    in_maps = []
    for r in range(N_CORES):
        shard = x[r * Bloc:(r + 1) * Bloc]
        in_maps.append(host_inputs(cfg, shard, W1, b1, W2, b2, W3, b3))

    res = run_bass_kernel_spmd(nc, in_maps, core_ids=list(range(N_CORES)),
                               **run_kwargs)
    outs = []
    for name in out_names:
        outs.append(np.concatenate([np.asarray(r[name]) for r in res.results],
                                   axis=1))
    return tuple(outs), res


def kernel(x, W1, b1, W2, b2, W3, b3):
    outs, _ = kernel_with_results(x, W1, b1, W2, b2, W3, b3)
    return outs


# revision 21
# speedup vs baseline: 82440.0210x; 82440.0210x over previous
"""Trainium2 Bass kernel for nn_Classifier (25-step LIF SNN, 784->64->64->10).

Strategy (pure data parallel over batch, 8 cores, Bloc=2048 rows/core):

Host side (numpy, inside kernel()): shard batch, pre-transpose x -> xT with
row-pair column ordering, build layout constants (W1^T, block-diagonal
W2^T / W3^T split into exact bf16 triples, identity, packed biases).

Device side per core, "row-pair packed feature-major" layout for layers
1/2: tensors are [128 partitions = 2 row-parities x 64 features, 1024
cols], where packed column c holds local batch rows (2c, 2c+1) - parity
on the partition blocks. This makes every transposed output partition
own two CONSECUTIVE batch rows, so output DMA runs are 512B contiguous
(full DMA-engine line rate instead of the sub-512B read-modify-write
penalty).

  phase 1:  x1 = W1 @ xT + b1 on the PE over 7 K-chunks (fp32), bias
            fused into the PSUM->SBUF copy on the scalar engine.
  step loop (t = 0..24), split into 2 column-chunks of 512:
    L1: m1' = beta*m1'' + x1 in ONE DVE scalar_tensor_tensor (bit-exact
        vs the reference), s1 = m1' > 1 (DVE, bf16 0/1), m1'' = m1' - s1.
    L2: q2 = W2bd @ s1 as three exact bf16 matmuls (spikes are binary so
        every product is exact; W2hi+W2mid+W2lo == W2 exactly in fp32),
        m2 = beta*m2'' + q2 (DVE STT), +b2 on ACT, spike + reset as L1.
    L3: q3 = W3bd @ s2 (bf16 triple), +b3 in the ACT PSUM->SBUF copy,
        transposed to batch-major, LIF recurrence on DVE.
    Outputs: m1'/m2' transposed to batch-major via PE transpose-mode;
    spikes (DVE is_gt) and sigmoids (ACT) stream to DRAM per step.
"""

from contextlib import ExitStack

import numpy as np

import concourse.bass as bass
import concourse.bacc as bacc
import concourse.mybir as mybir
import concourse.tile as tile

F32 = mybir.dt.float32
BF16 = mybir.dt.bfloat16
BETA = 0.95
THR = 1.0

# full-size problem constants
B_FULL = 16384
N_CORES = 8
KIN_FULL = 784
F_HID = 64
O_OUT = 10
STEPS_FULL = 25


class Cfg:
    def __init__(self, Bloc, Kin, steps, CH, out_steps=None):
        self.Bloc = Bloc          # local batch rows per core
        self.Kin = Kin            # input features (784)
        self.steps = steps
        # DRAM output depth; steps beyond this wrap (timing builds only)
        self.out_steps = out_steps or steps
        self.G = 2                # row-parity groups on partitions
        self.F = F_HID
        self.O = O_OUT
        self.P = self.G * self.F  # 128
        self.PO = self.G * self.O  # 20
        assert Bloc % self.G == 0
        self.COLS = Bloc // self.G   # packed columns
        self.CH = CH                 # column chunk per matmul/psum tile
        assert self.COLS % CH == 0
        self.NCH = self.COLS // CH
        self.TP = 128                # transpose block width (input cols)
        assert CH % self.TP == 0
        self.TC = CH // self.TP      # transpose blocks per chunk
        # K chunking for phase 1
        self.kcs = []
        k = Kin
        while k > 0:
            c = min(128, k)
            self.kcs.append(c)
            k -= c


def build_nc(cfg: Cfg):
    """Builds the per-core SPMD Bass program. Returns (nc, out_names)."""
    G, F, O, P, PO = cfg.G, cfg.F, cfg.O, cfg.P, cfg.PO
    COLS, CH, NCH, TP, TC = cfg.COLS, cfg.CH, cfg.NCH, cfg.TP, cfg.TC
    steps, Kin, Bloc = cfg.steps, cfg.Kin, cfg.Bloc
    GF = G * F
    GO = G * O
    AL = mybir.AluOpType
    SIG = mybir.ActivationFunctionType.Sigmoid

    nc = bacc.Bacc("TRN2", target_bir_lowering=False, debug=False,
                   enable_asserts=False)

    # ---- DRAM parameters (per core) ----
    xT_d = nc.declare_dram_parameter("xT", [Kin, Bloc], F32, isOutput=False)
    w1t_d = nc.declare_dram_parameter("w1t", [Kin, F], F32, isOutput=False)
    w2bd_d = nc.declare_dram_parameter("w2bd", [P, P], F32, isOutput=False)
    w3bd_d = nc.declare_dram_parameter("w3bd", [P, PO], F32, isOutput=False)
    eye_d = nc.declare_dram_parameter("eye", [P, P], F32, isOutput=False)
    b1_d = nc.declare_dram_parameter("b1p", [P, 1], F32, isOutput=False)
    b2_d = nc.declare_dram_parameter("b2p", [P, 1], F32, isOutput=False)
    b3_d = nc.declare_dram_parameter("b3p", [PO, 1], F32, isOutput=False)

    osteps = cfg.out_steps
    spko_d = nc.declare_dram_parameter("spk_out", [osteps, Bloc, O], F32, isOutput=True)
    spk1_d = nc.declare_dram_parameter("spk1", [osteps, Bloc, F], F32, isOutput=True)
    spk2_d = nc.declare_dram_parameter("spk2", [osteps, Bloc, F], F32, isOutput=True)
    ss1_d = nc.declare_dram_parameter("ss1", [osteps, Bloc, F], F32, isOutput=True)
    ss2_d = nc.declare_dram_parameter("ss2", [osteps, Bloc, F], F32, isOutput=True)
    ss3_d = nc.declare_dram_parameter("ss3", [osteps, Bloc, O], F32, isOutput=True)

    # batch-major DRAM views. Packed column c = j*TP + u (j = global
    # 128-col block) of parity g maps to local batch row r = 2*c + g, so
    # each transposed output partition owns two consecutive DRAM rows:
    # per-t APs are [c][j][(g f)] with a 2*f contiguous (512B) inner run.
    NJ = NCH * TC
    def bview(d):
        return d[:].rearrange("t (j c g) f -> t c j (g f)",
                              j=NJ, c=TP, g=G)

    v_spk1, v_spk2, v_ss1, v_ss2 = map(bview, (spk1_d, spk2_d, ss1_d, ss2_d))
    v_spko, v_ss3 = map(bview, (spko_d, ss3_d))

    with tile.TileContext(nc) as tc, ExitStack() as es:
        cpool = es.enter_context(tc.tile_pool(name="const", bufs=1))
        eye_s = cpool.tile([P, P], F32, tag="eye")
        w2bd_s = cpool.tile([P, P], F32, tag="w2bd")
        w3bd_s = cpool.tile([P, PO], F32, tag="w3bd")
        b1_s = cpool.tile([P, 1], F32, tag="b1")
        b2_s = cpool.tile([P, 1], F32, tag="b2")
        b3_s = cpool.tile([PO, 1], F32, tag="b3")
        nthr_s = cpool.tile([P, 1], F32, tag="nthr")
        x1f = cpool.tile([P, COLS], F32, tag="x1f")
        m3sb = cpool.tile([TP, NCH * TC * GO], F32, tag="m3sb")
        loads = [(eye_s, eye_d), (b1_s, b1_d), (b2_s, b2_d), (b3_s, b3_d),
                 (w2bd_s, w2bd_d), (w3bd_s, w3bd_d)]
        for t_s, t_d in loads:
            nc.sync.dma_start(t_s[:], t_d[:])
        nc.vector.memset(nthr_s[:], -THR)

        # ---------------- phase 1: x1 = W1 @ xT + b1 ----------------
        with tc.tile_pool(name="ph1", bufs=1) as xp, \
             tc.tile_pool(name="ph1ps", bufs=2, space="PSUM") as pp:
            xts, w1ts = [], []
            koff = 0
            for i, kc in enumerate(cfg.kcs):
                xt_t = xp.tile([kc, Bloc], F32, tag=f"xt{i}")
                nc.sync.dma_start(xt_t[:], xT_d[koff:koff + kc, :])
                w1_t = xp.tile([kc, F], F32, tag=f"w1{i}")
                nc.sync.dma_start(w1_t[:], w1t_d[koff:koff + kc, :])
                xts.append(xt_t)
                w1ts.append(w1_t)
                koff += kc
            for j in range(NCH):
                ps = pp.tile([P, CH], F32, tag="x1ps")
                for g in range(G):
                    for i, kc in enumerate(cfg.kcs):
                        # host pre-orders xT columns as [block j][parity g][u]
                        xv = xts[i][:].rearrange("p (j g u) -> p j g u",
                                                 g=G, u=TP)
                        nc.tensor.matmul(
                            ps[g * F:(g + 1) * F, :], w1ts[i][:],
                            xv[:, j * TC:(j + 1) * TC, g, :],
                            start=(i == 0), stop=(i == len(cfg.kcs) - 1))
                nc.scalar.add(x1f[:, j * CH:(j + 1) * CH], ps[:], b1_s[:])

        # ---------------- state pools ----------------
        sp = es.enter_context(tc.tile_pool(name="state", bufs=3))
        bp = es.enter_context(tc.tile_pool(name="bside", bufs=4))
        pq2 = es.enter_context(tc.tile_pool(name="pq2", bufs=2, space="PSUM"))
        pq3 = es.enter_context(tc.tile_pool(name="pq3", bufs=1, space="PSUM"))
        pt12 = es.enter_context(tc.tile_pool(name="pt12", bufs=2, space="PSUM"))
        pt3 = es.enter_context(tc.tile_pool(name="pt3", bufs=1, space="PSUM"))

        mm1_prev = sp.tile([P, COLS], F32, tag="mm1")
        mm2_prev = sp.tile([P, COLS], F32, tag="mm2")
        for z in (mm1_prev, mm2_prev):
            nc.vector.memset(z[:], 0.0)
        nc.vector.memset(m3sb[:], 0.0)

        for t_ in range(steps):
            t = t_ % osteps
            mm1_new = sp.tile([P, COLS], F32, tag="mm1")
            mm2_new = sp.tile([P, COLS], F32, tag="mm2")
            s1b = bp.tile([TP, NJ * GF], F32, tag="s1b", name="s1b")
            ss1b = bp.tile([TP, NJ * GF], F32, tag="ss1b", name="ss1b")
            s2b = bp.tile([TP, NJ * GF], F32, tag="s2b", name="s2b")
            ss2b = bp.tile([TP, NJ * GF], F32, tag="ss2b", name="ss2b")
            s3b = bp.tile([TP, NJ * GO], F32, tag="s3b", name="s3b")
            ss3b = bp.tile([TP, NJ * GO], F32, tag="ss3b", name="ss3b")
            tb1 = pt12.tile([TP, NJ * P], F32, tag="pt12", name="tb1")
            tb2 = pt12.tile([TP, NJ * P], F32, tag="pt12", name="tb2")
            t3 = pt3.tile([TP, NJ * PO], F32, tag="pt3", name="t3")
            for h in range(NCH):
                cs = slice(h * CH, (h + 1) * CH)
                # ---- L1 membrane: m1' = beta*m1'' + x1  (DVE, bit-exact)
                m1p = bp.tile([P, CH], F32, tag="m1p", name="m1p")
                nc.vector.scalar_tensor_tensor(
                    m1p[:], mm1_prev[:, cs], BETA, x1f[:, cs],
                    AL.mult, AL.add)
                s1f = bp.tile([P, CH], F32, tag="s1f", name="s1f")
                nc.vector.tensor_scalar(s1f[:], m1p[:], THR, None, AL.is_gt)
                nc.gpsimd.tensor_tensor(mm1_new[:, cs], m1p[:], s1f[:],
                                        AL.subtract)
                # ---- L2: q2 = W2 @ s1 (three exact bf16 passes)
                q2 = pq2.tile([P, CH], F32, tag="q2", name="q2")
                nc.tensor.matmul(q2[:], w2bd_s[:], s1f[:],
                                 start=True, stop=True)
                h2s = bp.tile([P, CH], F32, tag="h2s", name="h2s")
                nc.scalar.add(h2s[:], q2[:], b2_s[:])
                m2p = bp.tile([P, CH], F32, tag="m2p", name="m2p")
                nc.vector.scalar_tensor_tensor(
                    m2p[:], mm2_prev[:, cs], BETA, h2s[:], AL.mult, AL.add)
                s2f = bp.tile([P, CH], F32, tag="s2f", name="s2f")
                nc.vector.tensor_scalar(s2f[:], m2p[:], THR, None, AL.is_gt)
                nc.gpsimd.tensor_tensor(mm2_new[:, cs], m2p[:], s2f[:],
                                        AL.subtract)
                # ---- L3 feed-forward: q3 = W3 @ s2 (+b3 in ACT copy)
                q3 = pq3.tile([PO, CH], F32, tag="q3", name="q3")
                nc.tensor.matmul(q3[:], w3bd_s[:], s2f[:],
                                 start=True, stop=True)
                h3s = bp.tile([PO, CH], F32, tag="h3sb", name="h3s")
                nc.scalar.add(h3s[:], q3[:], b3_s[:])

                # ---- transposes into the per-step batch-major PSUM tiles
                for k in range(TC):
                    j = h * TC + k
                    nc.tensor.transpose(
                        tb1[:, j * P:(j + 1) * P],
                        m1p[:, k * TP:(k + 1) * TP], eye_s[:])
                    nc.tensor.transpose(
                        tb2[:, j * P:(j + 1) * P],
                        m2p[:, k * TP:(k + 1) * TP], eye_s[:])
                    nc.tensor.transpose(
                        t3[:, j * PO:(j + 1) * PO],
                        h3s[:, k * TP:(k + 1) * TP], eye_s[0:PO, 0:PO])

                # ---- L3 batch-major LIF (per half: columns of t3)
                ho = slice(h * TC * GO, (h + 1) * TC * GO)
                slab = m3sb[:, ho]
                m3t = bp.tile([TP, TC * GO], F32, tag="m3t", name="m3t")
                nc.vector.scalar_tensor_tensor(
                    m3t[:], slab, BETA,
                    t3[:, h * TC * PO:(h + 1) * TC * PO], AL.mult, AL.add)
                nc.gpsimd.tensor_scalar(s3b[:, ho], m3t[:], THR, None,
                                        AL.is_gt)
                nc.scalar.activation(ss3b[:, ho], m3t[:], SIG,
                                     bias=nthr_s[:])
                nc.gpsimd.tensor_tensor(slab, m3t[:], s3b[:, ho],
                                        AL.subtract)

            # ---- per-step batch-major spikes / sigmoids + one DMA each
            nc.vector.tensor_scalar(s1b[:], tb1[:], THR, None, AL.is_gt)
            nc.scalar.activation(ss1b[:], tb1[:], SIG, bias=nthr_s[:])
            nc.vector.tensor_scalar(s2b[:], tb2[:], THR, None, AL.is_gt)
            nc.scalar.activation(ss2b[:], tb2[:], SIG, bias=nthr_s[:])
            nc.sync.dma_start(v_spk1[t], s1b[:])
            nc.sync.dma_start(v_ss1[t], ss1b[:])
            nc.sync.dma_start(v_spk2[t], s2b[:])
            nc.sync.dma_start(v_ss2[t], ss2b[:])
            nc.sync.dma_start(v_spko[t], s3b[:])
            nc.sync.dma_start(v_ss3[t], ss3b[:])
            mm1_prev, mm2_prev = mm1_new, mm2_new

    nc.compile()
    out_names = ["spk_out", "spk1", "spk2", "ss1", "ss2", "ss3"]
    return nc, out_names


def host_inputs(cfg: Cfg, x_shard, W1, b1, W2, b2, W3, b3):
    """Builds the per-core input map (numpy data marshalling only)."""
    G, F, O, P, PO, TP = cfg.G, cfg.F, cfg.O, cfg.P, cfg.PO, cfg.TP
    f32 = np.float32
    x_shard = np.asarray(x_shard, f32)
    # column order [block j][parity g][u]: column (j*G+g)*TP+u holds batch
    # row 2*(j*TP+u)+g
    J = cfg.COLS // TP
    j = np.arange(J)[:, None, None]
    g = np.arange(G)[None, :, None]
    u = np.arange(TP)[None, None, :]
    idx = (2 * (j * TP + u) + g).reshape(-1)
    xT = np.ascontiguousarray(x_shard[idx].T)                # [Kin, Bloc]
    w1t = np.ascontiguousarray(np.asarray(W1, f32).T)        # [Kin, F]
    w2bd = np.zeros((P, P), f32)
    w2bd[0:F, 0:F] = np.asarray(W2, f32).T
    w2bd[F:P, F:P] = np.asarray(W2, f32).T
    w3bd = np.zeros((P, PO), f32)
    w3bd[0:F, 0:O] = np.asarray(W3, f32).T
    w3bd[F:P, O:PO] = np.asarray(W3, f32).T

    return {
        "xT": xT,
        "w1t": w1t,
        "w2bd": w2bd, "w3bd": w3bd,
        "b2p": np.tile(np.asarray(b2, f32), G).reshape(P, 1),
        "eye": np.eye(P, dtype=f32),
        "b1p": np.tile(np.asarray(b1, f32), G).reshape(P, 1),

        "b3p": np.tile(np.asarray(b3, f32), G).reshape(PO, 1),
    }


_CACHE = {}


def kernel_with_results(x, W1, b1, W2, b2, W3, b3, **run_kwargs):
    from concourse.bass_utils import run_bass_kernel_spmd

    x = np.asarray(x, np.float32)
    B = x.shape[0]
    assert B == B_FULL and x.shape[1] == KIN_FULL
    Bloc = B // N_CORES
    cfg = Cfg(Bloc=Bloc, Kin=KIN_FULL, steps=STEPS_FULL, CH=512)

    key = "full"
    if key not in _CACHE:
        _CACHE[key] = build_nc(cfg)
    nc, out_names = _CACHE[key]

    in_maps = []
    for r in range(N_CORES):
        shard = x[r * Bloc:(r + 1) * Bloc]
        in_maps.append(host_inputs(cfg, shard, W1, b1, W2, b2, W3, b3))

    res = run_bass_kernel_spmd(nc, in_maps, core_ids=list(range(N_CORES)),
                               **run_kwargs)
    outs = []
    for name in out_names:
        outs.append(np.concatenate([np.asarray(r[name]) for r in res.results],
                                   axis=1))
    return tuple(outs), res


def kernel(x, W1, b1, W2, b2, W3, b3):
    outs, _ = kernel_with_results(x, W1, b1, W2, b2, W3, b3)
    return outs
